# revision 5
# baseline (speedup 1.0000x reference)
"""Trainium2 Bass kernel for a single-step seq2seq GRU decoder with attention.

Computation (batch=1):
  embedded = emb[prev_word]                                      [1, H]
  attn_w   = softmax([embedded, h_prev] @ attn_W.T + attn_b)     [1, L]
  focused  = attn_w @ encoder_outputs                            [1, H]
  gru_in   = relu([focused, embedded] @ comb_W.T + comb_b)       [1, H]
  h_new    = GRU(gru_in, h_prev)                                 [1, H]
  probs    = log_softmax(h_new @ out_W.T + out_b)                [1, V]

Distribution over 8 NeuronCores (tensor parallel):
  - vocab dim of out_W/out_b sharded 8 ways (the 206MB input dominates)
  - comb_W / w_ih / w_hh sharded over their output (hidden) dim, with tiny
    AllGathers of gru_in and h_new between stages
  - attention stage replicated (tiny)
  - log-softmax via local exp-sums + scalar AllReduce

On-device layouts put the hidden/vocab dims on the 128 SBUF partitions:
a vector x[1024] lives as [128, 8] with x[c*128+p] at [p, c].
"""

import os
import numpy as np
import ml_dtypes

import concourse.bass as bass
import concourse.bacc as bacc
import concourse.mybir as mybir
import concourse.tile as tile
import concourse.bass_utils as bass_utils
from concourse.masks import make_identity

V, H, L = 50257, 1024, 20
N_CORES = 8
P = 128
KC = H // P            # 8 hidden chunks of 128
VP = 51200             # padded vocab = 8 * 6400
VS = VP // N_CORES     # 6400 vocab rows per core
VC = VS // P           # 50 vocab chunks of 128

F32 = mybir.dt.float32
BF16 = mybir.dt.bfloat16
NEG_BIG = -1.0e30

# dtype of the sharded weight matmuls (comb/GRU use WDT, out_W uses ODT)
WDT = BF16
ODT = BF16
WDT_NP = ml_dtypes.bfloat16 if WDT == BF16 else np.float32
ODT_NP = ml_dtypes.bfloat16 if ODT == BF16 else np.float32


def build_nc():
    nc = bacc.Bacc(
        "TRN2",
        target_bir_lowering=False,
        debug=False,
        enable_asserts=False,
        num_devices=N_CORES,
    )

    # ---- per-core external inputs -------------------------------------
    def inp(name, shape, dt=F32):
        return nc.dram_tensor(name, shape, dt, kind="ExternalInput").ap()

    emb_pc = inp("emb_pc", [P, KC])            # embedded word, (c p) -> p c
    hprev_pc = inp("hprev_pc", [P, KC])        # h_prev, (c p) -> p c
    hprev_slice = inp("hprev_slice", [P, 1])   # h_prev[128*i : 128*(i+1)]
    attn_wt = inp("attn_wt", [P, 16 * L])      # attn_W.T chunked [p, c*L+m]
    attn_b = inp("attn_b", [L, 1])
    enc = inp("enc", [L, H])                   # encoder_outputs
    comb_wt = inp("comb_wt", [P, 2 * H], WDT)  # comb_W shard .T chunked
    comb_b = inp("comb_b", [P, 1])
    wih_t = inp("wih_t", [P, 3 * H], WDT)      # w_ih shard .T [p,(g*KC+kc)*128+m]
    whh_t = inp("whh_t", [P, 3 * H], WDT)
    brz = inp("brz", [P, 2])                   # (b_ih+b_hh) r,z slices
    b_in = inp("b_in", [P, 1])                 # b_ih n slice
    b_hn = inp("b_hn", [P, 1])                 # b_hh n slice
    out_wt = inp("out_wt", [KC, P, VS], ODT)   # out_W shard .T chunks
    out_b = inp("out_b", [P, VC])              # bias shard [p, vc]

    probs_out = nc.dram_tensor("probs", [P, VC], F32, kind="ExternalOutput").ap()
    hnew_out = nc.dram_tensor("h_new", [KC, P], F32, kind="ExternalOutput").ap()

    with tile.TileContext(nc) as tc:
        with (
            tc.tile_pool(name="consts", bufs=1) as consts,
            tc.tile_pool(name="sb", bufs=1) as sb,
            tc.tile_pool(name="wpool", bufs=1) as wpool,
            tc.tile_pool(name="psum", bufs=1, space="PSUM") as psum,
            tc.tile_pool(name="dram", bufs=1, space="DRAM") as dram,
        ):
            # ---- constants ------------------------------------------------
            ones = consts.tile([P, P], F32)
            nc.vector.memset(ones[:], 1.0)
            ident = consts.tile([KC, KC], F32)
            make_identity(nc, ident[:])

            # ---- load small weights --------------------------------------
            emb_sb = sb.tile([P, KC], F32)
            nc.sync.dma_start(emb_sb[:], emb_pc)
            hprev_sb = sb.tile([P, KC], F32)
            nc.sync.dma_start(hprev_sb[:], hprev_pc)
            hprev_sl_sb = sb.tile([P, 1], F32)
            nc.sync.dma_start(hprev_sl_sb[:], hprev_slice)
            attn_wt_sb = sb.tile([P, 16 * L], F32)
            nc.sync.dma_start(attn_wt_sb[:], attn_wt)
            attn_b_sb = sb.tile([L, 1], F32)
            nc.sync.dma_start(attn_b_sb[:], attn_b)
            enc_sb = sb.tile([L, H], F32)
            nc.sync.dma_start(enc_sb[:], enc)
            comb_wt_sb = sb.tile([P, 2 * H], WDT)
            nc.sync.dma_start(comb_wt_sb[:], comb_wt)
            comb_b_sb = sb.tile([P, 1], F32)
            nc.sync.dma_start(comb_b_sb[:], comb_b)
            wih_sb = sb.tile([P, 3 * H], WDT)
            nc.sync.dma_start(wih_sb[:], wih_t)
            whh_sb = sb.tile([P, 3 * H], WDT)
            nc.sync.dma_start(whh_sb[:], whh_t)
            brz_sb = sb.tile([P, 2], F32)
            nc.sync.dma_start(brz_sb[:], brz)
            b_in_sb = sb.tile([P, 1], F32)
            nc.sync.dma_start(b_in_sb[:], b_in)
            b_hn_sb = sb.tile([P, 1], F32)
            nc.sync.dma_start(b_hn_sb[:], b_hn)
            out_b_sb = sb.tile([P, VC], F32)
            nc.sync.dma_start(out_b_sb[:], out_b)

            # ---- big out_W shard: one tile per hidden chunk --------------
            w_sb = []
            for kc in range(KC):
                wt = wpool.tile([P, VS], ODT, tag=f"w{kc}")
                half = VS // 2
                nc.sync.dma_start(wt[:, :half], out_wt[kc][:, :half])
                nc.sync.dma_start(wt[:, half:], out_wt[kc][:, half:])
                w_sb.append(wt)

            # ---- stage A: attention (replicated, fp32) -------------------
            # attn logits [L,1] across 16 k-chunks of [embedded, h_prev]
            attnlog_ps = psum.tile([L, 1], F32, tag="pa")
            for c in range(16):
                rhs = emb_sb[:, c : c + 1] if c < KC else hprev_sb[:, c - KC : c - KC + 1]
                nc.tensor.matmul(
                    attnlog_ps[:],
                    attn_wt_sb[:, c * L : (c + 1) * L],
                    rhs,
                    start=(c == 0),
                    stop=(c == 15),
                )
            # exp(logit + b); values are tiny so no max-subtraction needed
            expw_sb = sb.tile([L, 1], F32)
            nc.scalar.activation(
                expw_sb[:], attnlog_ps[:], mybir.ActivationFunctionType.Exp,
                bias=attn_b_sb[:],
            )
            # denom = sum over the 20 partitions (PE with ones vector)
            asum_ps = psum.tile([1, 1], F32, tag="pb")
            nc.tensor.matmul(asum_ps[:], expw_sb[:], ones[:L, 0:1], start=True, stop=True)
            arecip_sb = sb.tile([1, 1], F32)
            nc.vector.reciprocal(arecip_sb[:], asum_ps[:])
            # focused (unnormalized) [128, KC]: chunk c = enc[:, c*128:+128].T @ expw
            foc_ps = psum.tile([P, KC], F32, tag="pm")
            for c in range(KC):
                nc.tensor.matmul(
                    foc_ps[:, c : c + 1],
                    enc_sb[:, c * P : (c + 1) * P],
                    expw_sb[:],
                    start=True,
                    stop=True,
                )
            # broadcast 1/denom across partitions via PE ones column
            arb_ps = psum.tile([P, 1], F32, tag="pb")
            nc.tensor.matmul(arb_ps[:], ones[0:1, :], arecip_sb[:], start=True, stop=True)
            arb_sb = sb.tile([P, 1], F32)
            nc.scalar.copy(arb_sb[:], arb_ps[:])
            # normalized focused, cast to WDT for the comb matmul
            fsc_sb = sb.tile([P, KC], WDT)
            nc.vector.tensor_scalar_mul(fsc_sb[:], foc_ps[:], arb_sb[:])
            emb_w_sb = sb.tile([P, KC], WDT)
            nc.vector.tensor_copy(emb_w_sb[:], emb_sb[:])

            # ---- stage B: comb shard -> gru_in slice [128,1] -------------
            gcol_ps = psum.tile([P, 1], F32, tag="pb")
            for c in range(16):
                rhs = fsc_sb[:, c : c + 1] if c < KC else emb_w_sb[:, c - KC : c - KC + 1]
                nc.tensor.matmul(
                    gcol_ps[:],
                    comb_wt_sb[:, c * P : (c + 1) * P],
                    rhs,
                    start=(c == 0),
                    stop=(c == 15),
                )
            gcol_sb = sb.tile([P, 1], F32)
            nc.scalar.activation(
                gcol_sb[:], gcol_ps[:], mybir.ActivationFunctionType.Relu,
                bias=comb_b_sb[:],
            )

            # ---- all-gather gru_in ---------------------------------------
            g1_in = dram.tile([P, 1], F32)
            g1_out = dram.tile([KC, P], F32)
            nc.sync.dma_start(g1_in[:], gcol_sb[:])
            nc.gpsimd.collective_compute(
                "AllGather",
                mybir.AluOpType.bypass,
                replica_groups=[list(range(N_CORES))],
                ins=[g1_in.opt()],
                outs=[g1_out.opt()],
            )
            g1_sb = sb.tile([KC, P], F32)
            nc.sync.dma_start(g1_sb[:], g1_out[:])
            gin_t_ps = psum.tile([P, KC], F32, tag="pm")
            nc.tensor.transpose(gin_t_ps[:], g1_sb[:], ident[:])
            gin_sb = sb.tile([P, KC], WDT)
            nc.vector.tensor_copy(gin_sb[:], gin_t_ps[:])
            hprev_w_sb = sb.tile([P, KC], WDT)
            nc.vector.tensor_copy(hprev_w_sb[:], hprev_sb[:])

            # ---- stage C: GRU shard -> h_new slice [128,1] ---------------
            gi_ps = psum.tile([P, 3], F32, tag="pgi")
            gh_ps = psum.tile([P, 3], F32, tag="pgh")
            for g in range(3):
                for kc in range(KC):
                    lhs = wih_sb[:, (g * KC + kc) * P : (g * KC + kc + 1) * P]
                    nc.tensor.matmul(
                        gi_ps[:, g : g + 1], lhs, gin_sb[:, kc : kc + 1],
                        start=(kc == 0), stop=(kc == KC - 1),
                    )
            for g in range(3):
                for kc in range(KC):
                    lhs = whh_sb[:, (g * KC + kc) * P : (g * KC + kc + 1) * P]
                    nc.tensor.matmul(
                        gh_ps[:, g : g + 1], lhs, hprev_w_sb[:, kc : kc + 1],
                        start=(kc == 0), stop=(kc == KC - 1),
                    )
            gh_sb = sb.tile([P, 3], F32)
            nc.scalar.copy(gh_sb[:], gh_ps[:])
            rz_sb = sb.tile([P, 2], F32)
            nc.vector.tensor_add(rz_sb[:], gi_ps[:, 0:2], gh_sb[:, 0:2])
            r_sb = sb.tile([P, 1], F32)
            nc.scalar.activation(
                r_sb[:], rz_sb[:, 0:1], mybir.ActivationFunctionType.Sigmoid,
                bias=brz_sb[:, 0:1],
            )
            z_sb = sb.tile([P, 1], F32)
            nc.scalar.activation(
                z_sb[:], rz_sb[:, 1:2], mybir.ActivationFunctionType.Sigmoid,
                bias=brz_sb[:, 1:2],
            )
            # n = tanh(in + b_in + r*(hn + b_hn))
            npre_sb = sb.tile([P, 1], F32)
            # (gh_n + b_hn) * r
            nc.vector.scalar_tensor_tensor(
                npre_sb[:], gh_sb[:, 2:3], b_hn_sb[:], r_sb[:],
                op0=mybir.AluOpType.add, op1=mybir.AluOpType.mult,
            )
            nc.vector.tensor_add(npre_sb[:], npre_sb[:], gi_ps[:, 2:3])
            n_sb = sb.tile([P, 1], F32)
            nc.scalar.activation(
                n_sb[:], npre_sb[:], mybir.ActivationFunctionType.Tanh,
                bias=b_in_sb[:],
            )
            # h_new = n + z*(h - n)
            hmn_sb = sb.tile([P, 1], F32)
            nc.vector.tensor_sub(hmn_sb[:], hprev_sl_sb[:], n_sb[:])
            hnew_col_sb = sb.tile([P, 1], F32)
            nc.vector.scalar_tensor_tensor(
                hnew_col_sb[:], hmn_sb[:], 1.0, z_sb[:],
                op0=mybir.AluOpType.mult, op1=mybir.AluOpType.mult,
            )
            nc.vector.tensor_add(hnew_col_sb[:], hnew_col_sb[:], n_sb[:])

            # ---- all-gather h_new ----------------------------------------
            g2_in = dram.tile([P, 1], F32)
            g2_out = dram.tile([KC, P], F32)
            nc.sync.dma_start(g2_in[:], hnew_col_sb[:])
            nc.gpsimd.collective_compute(
                "AllGather",
                mybir.AluOpType.bypass,
                replica_groups=[list(range(N_CORES))],
                ins=[g2_in.opt()],
                outs=[g2_out.opt()],
            )
            nc.sync.dma_start(hnew_out, g2_out[:])
            g2_sb = sb.tile([KC, P], F32)
            nc.sync.dma_start(g2_sb[:], g2_out[:])
            hn_t_ps = psum.tile([P, KC], F32, tag="pm")
            nc.tensor.transpose(hn_t_ps[:], g2_sb[:], ident[:])
            hnew_sb = sb.tile([P, KC], ODT)
            nc.vector.tensor_copy(hnew_sb[:], hn_t_ps[:])

            # ---- stage D: vocab projection shard -------------------------
            # Per hidden-chunk partial products in PSUM, accumulated into
            # SBUF by DVE. Keeps PE free to start on each weight chunk as
            # its DMA lands, with no cross-chunk PSUM accumulation groups.
            logits_sb = sb.tile([P, VC], F32)
            for kc in range(KC):
                part_ps = psum.tile([P, VC], F32, tag="plog", bufs=2)
                for vc in range(VC):
                    nc.tensor.matmul(
                        part_ps[:, vc : vc + 1],
                        w_sb[kc][:, vc * P : (vc + 1) * P],
                        hnew_sb[:, kc : kc + 1],
                        start=True,
                        stop=True,
                    )
                if kc == 0:
                    nc.vector.tensor_add(logits_sb[:], part_ps[:], out_b_sb[:])
                else:
                    nc.vector.tensor_add(logits_sb[:], logits_sb[:], part_ps[:])

            # ---- sharded log-softmax -------------------------------------
            exp_sb = sb.tile([P, VC], F32)
            srow_sb = sb.tile([P, 1], F32)
            nc.scalar.activation(
                exp_sb[:], logits_sb[:], mybir.ActivationFunctionType.Exp,
                accum_out=srow_sb[:],
            )
            ssum_ps = psum.tile([1, 1], F32, tag="pb")
            nc.tensor.matmul(ssum_ps[:], srow_sb[:], ones[:, 0:1], start=True, stop=True)
            s_sb = sb.tile([1, 1], F32)
            nc.scalar.copy(s_sb[:], ssum_ps[:])
            g3_in = dram.tile([1, 1], F32)
            g3_out = dram.tile([1, 1], F32)
            nc.sync.dma_start(g3_in[:], s_sb[:])
            nc.gpsimd.collective_compute(
                "AllReduce",
                mybir.AluOpType.add,
                replica_groups=[list(range(N_CORES))],
                ins=[g3_in.opt()],
                outs=[g3_out.opt()],
            )
            stot_sb = sb.tile([1, 1], F32)
            nc.sync.dma_start(stot_sb[:], g3_out[:])
            logs_sb = sb.tile([1, 1], F32)
            nc.scalar.activation(
                logs_sb[:], stot_sb[:], mybir.ActivationFunctionType.Ln,
            )
            lsb_ps = psum.tile([P, 1], F32, tag="pb")
            nc.tensor.matmul(lsb_ps[:], ones[0:1, :], logs_sb[:], start=True, stop=True)
            lsb_sb = sb.tile([P, 1], F32)
            nc.scalar.copy(lsb_sb[:], lsb_ps[:])
            probs_sb = sb.tile([P, VC], F32)
            nc.vector.tensor_scalar_sub(probs_sb[:], logits_sb[:], lsb_sb[:])
            nc.sync.dma_start(probs_out, probs_sb[:])

    nc.compile()
    return nc


_NC_CACHE = None


def _get_nc():
    global _NC_CACHE
    if _NC_CACHE is None:
        _NC_CACHE = build_nc()
    return _NC_CACHE


def _pc(v):
    """[1024] -> [128, 8] with v[c*128+p] at [p, c]."""
    return np.ascontiguousarray(v.reshape(KC, P).T)


def make_in_maps(prev_word, prev_hidden, encoder_outputs, emb, attn_W, attn_b,
                 comb_W, comb_b, w_ih, w_hh, b_ih, b_hh, out_W, out_b):
    f32 = lambda a: np.asarray(a, dtype=np.float32)
    idx = int(np.asarray(prev_word).reshape(-1)[0])
    emb_row = f32(emb)[idx].reshape(H)
    hprev = f32(prev_hidden).reshape(H)
    attn_W = f32(attn_W)
    attn_b = f32(attn_b)
    enc = np.ascontiguousarray(f32(encoder_outputs))
    comb_W = f32(comb_W)
    comb_b = f32(comb_b)
    w_ih, w_hh, b_ih, b_hh = f32(w_ih), f32(w_hh), f32(b_ih), f32(b_hh)
    out_W, out_b = f32(out_W), f32(out_b)

    emb_pc = _pc(emb_row)
    hprev_pc = _pc(hprev)
    # attn_W.T chunked: [p, c*L+m] = attn_W[m, c*128+p]
    attn_wt = np.ascontiguousarray(
        attn_W.T.reshape(16, P, L).transpose(1, 0, 2).reshape(P, 16 * L))
    attn_b_c = np.ascontiguousarray(attn_b.reshape(L, 1))

    Wp = np.zeros((VP, H), np.float32)
    Wp[:V] = out_W
    bp = np.full(VP, NEG_BIG, np.float32)
    bp[:V] = out_b

    in_maps = []
    for i in range(N_CORES):
        hsl = slice(P * i, P * (i + 1))
        # comb_W shard .T chunked: [p, c*128+m] = comb_W[128i+m, c*128+p]
        comb_wt = np.ascontiguousarray(
            comb_W[hsl].T.reshape(16, P, P).transpose(1, 0, 2).reshape(P, 2 * H)
        ).astype(WDT_NP)
        grow = lambda W, g: W[H * g + P * i: H * g + P * (i + 1)]
        # [p, (g*KC+kc)*128+m] = W[1024g+128i+m, kc*128+p]
        gru_t = lambda W: np.ascontiguousarray(
            np.stack([grow(W, g).T.reshape(KC, P, P) for g in range(3)])
            .transpose(2, 0, 1, 3).reshape(P, 3 * H)).astype(WDT_NP)
        bsum = b_ih + b_hh
        brz = np.stack([bsum[0 * H + P * i: 0 * H + P * (i + 1)],
                        bsum[1 * H + P * i: 1 * H + P * (i + 1)]], axis=1)
        b_in_c = b_ih[2 * H + P * i: 2 * H + P * (i + 1)].reshape(P, 1)
        b_hn_c = b_hh[2 * H + P * i: 2 * H + P * (i + 1)].reshape(P, 1)

        vsl = slice(VS * i, VS * (i + 1))
        out_wt = np.ascontiguousarray(Wp[vsl].T.reshape(KC, P, VS)).astype(ODT_NP)
        out_b_c = np.ascontiguousarray(bp[vsl].reshape(VC, P).T)

        in_maps.append({
            "emb_pc": emb_pc,
            "hprev_pc": hprev_pc,
            "hprev_slice": np.ascontiguousarray(hprev[hsl].reshape(P, 1)),
            "attn_wt": attn_wt,
            "attn_b": attn_b_c,
            "enc": enc,
            "comb_wt": comb_wt,
            "comb_b": comb_b[hsl].reshape(P, 1),
            "wih_t": gru_t(w_ih),
            "whh_t": gru_t(w_hh),
            "brz": np.ascontiguousarray(brz),
            "b_in": b_in_c,
            "b_hn": b_hn_c,
            "out_wt": out_wt,
            "out_b": out_b_c,
        })
    return in_maps


LAST_RESULTS = None


def kernel(**inputs):
    global LAST_RESULTS
    nc = _get_nc()
    in_maps = make_in_maps(**inputs)
    trace = bool(int(os.environ.get("KERNEL_TRACE", "0")))
    res = bass_utils.run_bass_kernel_spmd(
        nc, in_maps, core_ids=list(range(N_CORES)), trace=trace,
    )
    LAST_RESULTS = res
    probs = np.concatenate(
        [np.asarray(r["probs"]).T.reshape(VS) for r in res.results])[:V]
    h_new = np.asarray(res.results[0]["h_new"]).reshape(1, 1, H)
    return probs.reshape(1, V).astype(np.float32), h_new.astype(np.float32)


# revision 8
# speedup vs baseline: 1.6497x; 1.6497x over previous
"""Trainium2 Bass kernel for a single-step seq2seq GRU decoder with attention.

Computation (batch=1):
  embedded = emb[prev_word]                                      [1, H]
  attn_w   = softmax([embedded, h_prev] @ attn_W.T + attn_b)     [1, L]
  focused  = attn_w @ encoder_outputs                            [1, H]
  gru_in   = relu([focused, embedded] @ comb_W.T + comb_b)       [1, H]
  h_new    = GRU(gru_in, h_prev)                                 [1, H]
  probs    = log_softmax(h_new @ out_W.T + out_b)                [1, V]

Distribution over 8 NeuronCores, with NO cross-core collectives:
  - the vocab dim of out_W/out_b (the 206MB input that dominates the
    memory-bound roofline) is sharded 8 ways
  - the small attention/comb/GRU stages are replicated on every core
    (batch=1: cheaper than paying a cross-core sync for their shards)
  - log-softmax: each core emits its local exp-sum; the host unshard step
    combines the 8 scalars and subtracts log(S) while concatenating.
  Collectives are deliberately avoided: a NEFF with collectives pays a
  multi-core rendezvous at entry, which costs the full inter-core dispatch
  skew on every execution.

On-device layout: a hidden vector x[1024] lives as [128, 8] SBUF tiles with
x[c*128+p] at [p, c] (partition-parallel everywhere; no transposes needed).
"""

import os
import numpy as np
import ml_dtypes

import concourse.bass as bass
import concourse.bacc as bacc
import concourse.mybir as mybir
import concourse.tile as tile
import concourse.bass_utils as bass_utils

V, H, L = 50257, 1024, 20
N_CORES = 8
P = 128
KC = H // P            # 8 hidden chunks of 128
VP = 51200             # padded vocab = 8 * 6400
VS = VP // N_CORES     # 6400 vocab rows per core
VC = VS // P           # 50 vocab chunks of 128

F32 = mybir.dt.float32
BF16 = mybir.dt.bfloat16
NEG_BIG = -1.0e30

# dtype of the replicated comb/GRU weights (WDT) and the out_W shard (ODT)
WDT = BF16
ODT = BF16
WDT_NP = ml_dtypes.bfloat16 if WDT == BF16 else np.float32
ODT_NP = ml_dtypes.bfloat16 if ODT == BF16 else np.float32


def build_nc():
    nc = bacc.Bacc(
        "TRN2",
        target_bir_lowering=False,
        debug=False,
        enable_asserts=False,
        num_devices=N_CORES,
    )

    def inp(name, shape, dt=F32):
        return nc.dram_tensor(name, shape, dt, kind="ExternalInput").ap()

    # replicated inputs
    emb_pc = inp("emb_pc", [P, KC])            # embedded word, (c p) -> p c
    hprev_pc = inp("hprev_pc", [P, KC])        # h_prev, (c p) -> p c
    attn_wt = inp("attn_wt", [P, 16 * L])      # attn_W.T chunked [p, c*L+m]
    attn_b = inp("attn_b", [L, 1])
    enc = inp("enc", [L, H])                   # encoder_outputs
    comb_wt = inp("comb_wt", [P, 16 * KC * P], WDT)  # [p,(co*16+ck)*128+m]
    comb_b = inp("comb_b", [P, KC])
    wih_t = inp("wih_t", [P, 3 * KC * H], WDT)  # [p,((g*8+co)*8+kc)*128+m]
    whh_t = inp("whh_t", [P, 3 * KC * H], WDT)
    brz = inp("brz", [P, 2 * KC])              # (b_ih+b_hh) r,z in (c p)
    b_in = inp("b_in", [P, KC])                # b_ih n slice
    b_hn = inp("b_hn", [P, KC])                # b_hh n slice
    # sharded inputs
    out_wt = inp("out_wt", [KC, P, VS], ODT)   # out_W shard .T chunks
    out_b = inp("out_b", [P, VC])              # bias shard [p, vc]

    logits_out = nc.dram_tensor("logits", [P, VC], F32, kind="ExternalOutput").ap()
    ssum_out = nc.dram_tensor("ssum", [1, 1], F32, kind="ExternalOutput").ap()
    hnew_out = nc.dram_tensor("h_new", [P, KC], F32, kind="ExternalOutput").ap()

    with tile.TileContext(nc) as tc:
        with (
            tc.tile_pool(name="consts", bufs=1) as consts,
            tc.tile_pool(name="sb", bufs=1) as sb,
            tc.tile_pool(name="wpool", bufs=1) as wpool,
            tc.tile_pool(name="psum", bufs=1, space="PSUM") as psum,
        ):
            ones = consts.tile([P, P], F32)
            nc.vector.memset(ones[:], 1.0)

            # ---- small/critical weights first (DMA order matters) --------
            emb_sb = sb.tile([P, KC], F32)
            nc.sync.dma_start(emb_sb[:], emb_pc)
            hprev_sb = sb.tile([P, KC], F32)
            nc.sync.dma_start(hprev_sb[:], hprev_pc)
            attn_wt_sb = sb.tile([P, 16 * L], F32)
            nc.sync.dma_start(attn_wt_sb[:], attn_wt)
            attn_b_sb = sb.tile([L, 1], F32)
            nc.sync.dma_start(attn_b_sb[:], attn_b)
            enc_sb = sb.tile([L, H], F32)
            nc.sync.dma_start(enc_sb[:], enc)
            comb_b_sb = sb.tile([P, KC], F32)
            nc.sync.dma_start(comb_b_sb[:], comb_b)
            brz_sb = sb.tile([P, 2 * KC], F32)
            nc.sync.dma_start(brz_sb[:], brz)
            b_in_sb = sb.tile([P, KC], F32)
            nc.sync.dma_start(b_in_sb[:], b_in)
            b_hn_sb = sb.tile([P, KC], F32)
            nc.sync.dma_start(b_hn_sb[:], b_hn)
            out_b_sb = sb.tile([P, VC], F32)
            nc.sync.dma_start(out_b_sb[:], out_b)

            comb_wt_sb = sb.tile([P, 16 * KC * P], WDT)
            nc.sync.dma_start(comb_wt_sb[:], comb_wt)
            wih_sb = sb.tile([P, 3 * KC * H], WDT)
            half = 3 * KC * H // 2
            nc.sync.dma_start(wih_sb[:, :half], wih_t[:, :half])
            nc.sync.dma_start(wih_sb[:, half:], wih_t[:, half:])
            whh_sb = sb.tile([P, 3 * KC * H], WDT)
            nc.sync.dma_start(whh_sb[:, :half], whh_t[:, :half])
            nc.sync.dma_start(whh_sb[:, half:], whh_t[:, half:])

            # ---- big out_W shard: 4-deep rotating stream of chunks -------
            # (all 8 resident would not fit next to the replicated GRU
            # weights; DMA refills slots as PE drains them)
            def load_w_chunk(kc):
                wt = wpool.tile([P, VS], ODT, tag="w", bufs=4)
                vh = VS // 2
                nc.sync.dma_start(wt[:, :vh], out_wt[kc][:, :vh])
                nc.sync.dma_start(wt[:, vh:], out_wt[kc][:, vh:])
                return wt

            # ---- stage A: attention (replicated, fp32) -------------------
            attnlog_ps = psum.tile([L, 1], F32, tag="pa")
            for c in range(16):
                rhs = emb_sb[:, c : c + 1] if c < KC else hprev_sb[:, c - KC : c - KC + 1]
                nc.tensor.matmul(
                    attnlog_ps[:],
                    attn_wt_sb[:, c * L : (c + 1) * L],
                    rhs,
                    start=(c == 0),
                    stop=(c == 15),
                )
            # exp(logit + b); logits are tiny so no max-subtraction needed
            expw_sb = sb.tile([L, 1], F32)
            nc.scalar.activation(
                expw_sb[:], attnlog_ps[:], mybir.ActivationFunctionType.Exp,
                bias=attn_b_sb[:],
            )
            asum_ps = psum.tile([1, 1], F32, tag="pb")
            nc.tensor.matmul(asum_ps[:], expw_sb[:], ones[:L, 0:1], start=True, stop=True)
            arecip_sb = sb.tile([1, 1], F32)
            nc.vector.reciprocal(arecip_sb[:], asum_ps[:])
            # focused (unnormalized) [128, KC]
            foc_ps = psum.tile([P, KC], F32, tag="pm")
            for c in range(KC):
                nc.tensor.matmul(
                    foc_ps[:, c : c + 1],
                    enc_sb[:, c * P : (c + 1) * P],
                    expw_sb[:],
                    start=True,
                    stop=True,
                )
            # broadcast 1/denom across partitions via PE ones column
            arb_ps = psum.tile([P, 1], F32, tag="pb")
            nc.tensor.matmul(arb_ps[:], ones[0:1, :], arecip_sb[:], start=True, stop=True)
            arb_sb = sb.tile([P, 1], F32)
            nc.scalar.copy(arb_sb[:], arb_ps[:])
            fsc_sb = sb.tile([P, KC], WDT)
            nc.vector.tensor_scalar_mul(fsc_sb[:], foc_ps[:], arb_sb[:])
            emb_w_sb = sb.tile([P, KC], WDT)
            nc.vector.tensor_copy(emb_w_sb[:], emb_sb[:])

            # ---- stage B: comb (replicated) -> gru_in [128, KC] ----------
            gcol_ps = psum.tile([P, KC], F32, tag="pm")
            for co in range(KC):
                for ck in range(16):
                    rhs = (fsc_sb[:, ck : ck + 1] if ck < KC
                           else emb_w_sb[:, ck - KC : ck - KC + 1])
                    nc.tensor.matmul(
                        gcol_ps[:, co : co + 1],
                        comb_wt_sb[:, (co * 16 + ck) * P : (co * 16 + ck + 1) * P],
                        rhs,
                        start=(ck == 0),
                        stop=(ck == 15),
                    )
            gin_f_sb = sb.tile([P, KC], F32)
            nc.vector.tensor_add(gin_f_sb[:], gcol_ps[:], comb_b_sb[:])
            nc.vector.tensor_relu(gin_f_sb[:], gin_f_sb[:])
            gin_sb = sb.tile([P, KC], WDT)
            nc.vector.tensor_copy(gin_sb[:], gin_f_sb[:])
            hprev_w_sb = sb.tile([P, KC], WDT)
            nc.vector.tensor_copy(hprev_w_sb[:], hprev_sb[:])

            # ---- stage C: GRU (replicated) -> h_new [128, KC] ------------
            # gi/gh columns j = g*KC + co (gate-major)
            gi_ps = psum.tile([P, 3 * KC], F32, tag="pgi")
            gh_ps = psum.tile([P, 3 * KC], F32, tag="pgh")
            for j in range(3 * KC):
                for kc in range(KC):
                    off = (j * KC + kc) * P
                    nc.tensor.matmul(
                        gi_ps[:, j : j + 1],
                        wih_sb[:, off : off + P],
                        gin_sb[:, kc : kc + 1],
                        start=(kc == 0),
                        stop=(kc == KC - 1),
                    )
            for j in range(3 * KC):
                for kc in range(KC):
                    off = (j * KC + kc) * P
                    nc.tensor.matmul(
                        gh_ps[:, j : j + 1],
                        whh_sb[:, off : off + P],
                        hprev_w_sb[:, kc : kc + 1],
                        start=(kc == 0),
                        stop=(kc == KC - 1),
                    )
            gh_sb = sb.tile([P, 3 * KC], F32)
            nc.scalar.copy(gh_sb[:], gh_ps[:])
            # r,z = sigmoid(gi + gh + brz) on the first 2*KC columns
            rz_sb = sb.tile([P, 2 * KC], F32)
            nc.vector.tensor_add(rz_sb[:], gi_ps[:, : 2 * KC], gh_sb[:, : 2 * KC])
            nc.vector.tensor_add(rz_sb[:], rz_sb[:], brz_sb[:])
            nc.scalar.activation(rz_sb[:], rz_sb[:], mybir.ActivationFunctionType.Sigmoid)
            # n = tanh(gi_n + b_in + r*(gh_n + b_hn))
            hnb_sb = sb.tile([P, KC], F32)
            nc.vector.tensor_add(hnb_sb[:], gh_sb[:, 2 * KC :], b_hn_sb[:])
            nc.vector.tensor_mul(hnb_sb[:], hnb_sb[:], rz_sb[:, :KC])
            npre_sb = sb.tile([P, KC], F32)
            nc.vector.tensor_add(npre_sb[:], gi_ps[:, 2 * KC :], hnb_sb[:])
            nc.vector.tensor_add(npre_sb[:], npre_sb[:], b_in_sb[:])
            n_sb = sb.tile([P, KC], F32)
            nc.scalar.activation(n_sb[:], npre_sb[:], mybir.ActivationFunctionType.Tanh)
            # h_new = n + z*(h - n)
            hmn_sb = sb.tile([P, KC], F32)
            nc.vector.tensor_sub(hmn_sb[:], hprev_sb[:], n_sb[:])
            nc.vector.tensor_mul(hmn_sb[:], hmn_sb[:], rz_sb[:, KC : 2 * KC])
            hnew_sb = sb.tile([P, KC], F32)
            nc.vector.tensor_add(hnew_sb[:], n_sb[:], hmn_sb[:])
            nc.sync.dma_start(hnew_out, hnew_sb[:])
            hnew_w_sb = sb.tile([P, KC], ODT)
            nc.vector.tensor_copy(hnew_w_sb[:], hnew_sb[:])

            # ---- stage D: vocab projection shard -------------------------
            # Per hidden-chunk partials in PSUM, accumulated into SBUF by
            # DVE: PE starts on each weight chunk as its DMA lands, no
            # cross-chunk PSUM accumulation groups.
            logits_sb = sb.tile([P, VC], F32)
            for kc in range(KC):
                wt = load_w_chunk(kc)
                part_ps = psum.tile([P, VC], F32, tag="plog", bufs=2)
                for vc in range(VC):
                    nc.tensor.matmul(
                        part_ps[:, vc : vc + 1],
                        wt[:, vc * P : (vc + 1) * P],
                        hnew_w_sb[:, kc : kc + 1],
                        start=True,
                        stop=True,
                    )
                if kc == 0:
                    nc.vector.tensor_add(logits_sb[:], part_ps[:], out_b_sb[:])
                else:
                    nc.vector.tensor_add(logits_sb[:], logits_sb[:], part_ps[:])
            nc.sync.dma_start(logits_out, logits_sb[:])

            # ---- local exp-sum for the host-side log-softmax -------------
            exp_sb = sb.tile([P, VC], F32)
            srow_sb = sb.tile([P, 1], F32)
            nc.scalar.activation(
                exp_sb[:], logits_sb[:], mybir.ActivationFunctionType.Exp,
                accum_out=srow_sb[:],
            )
            ssum_ps = psum.tile([1, 1], F32, tag="pb")
            nc.tensor.matmul(ssum_ps[:], srow_sb[:], ones[:, 0:1], start=True, stop=True)
            ssum_sb = sb.tile([1, 1], F32)
            nc.scalar.copy(ssum_sb[:], ssum_ps[:])
            nc.sync.dma_start(ssum_out, ssum_sb[:])

    nc.compile()
    return nc


_NC_CACHE = None


def _get_nc():
    global _NC_CACHE
    if _NC_CACHE is None:
        _NC_CACHE = build_nc()
    return _NC_CACHE


def _pc(v):
    """[1024] -> [128, 8] with v[c*128+p] at [p, c]."""
    return np.ascontiguousarray(v.reshape(KC, P).T)


def make_in_maps(prev_word, prev_hidden, encoder_outputs, emb, attn_W, attn_b,
                 comb_W, comb_b, w_ih, w_hh, b_ih, b_hh, out_W, out_b):
    f32 = lambda a: np.asarray(a, dtype=np.float32)
    idx = int(np.asarray(prev_word).reshape(-1)[0])
    emb_row = f32(emb)[idx].reshape(H)
    hprev = f32(prev_hidden).reshape(H)
    attn_W = f32(attn_W)
    attn_b = f32(attn_b)
    enc = np.ascontiguousarray(f32(encoder_outputs))
    comb_W = f32(comb_W)
    comb_b = f32(comb_b)
    w_ih, w_hh, b_ih, b_hh = f32(w_ih), f32(w_hh), f32(b_ih), f32(b_hh)
    out_W, out_b = f32(out_W), f32(out_b)

    # replicated tensors (same arrays for every core)
    rep = {
        "emb_pc": _pc(emb_row),
        "hprev_pc": _pc(hprev),
        # [p, c*L+m] = attn_W[m, c*128+p]
        "attn_wt": np.ascontiguousarray(
            attn_W.T.reshape(16, P, L).transpose(1, 0, 2).reshape(P, 16 * L)),
        "attn_b": np.ascontiguousarray(attn_b.reshape(L, 1)),
        "enc": enc,
        # [p, (co*16+ck)*128+m] = comb_W[co*128+m, ck*128+p]
        "comb_wt": np.ascontiguousarray(
            comb_W.reshape(KC, P, 16, P).transpose(3, 0, 2, 1)
            .reshape(P, 16 * KC * P)).astype(WDT_NP),
        "comb_b": _pc(comb_b),
        # [p, ((g*8+co)*8+kc)*128+m] = W[g*1024+co*128+m, kc*128+p]
        "wih_t": np.ascontiguousarray(
            w_ih.reshape(3, KC, P, KC, P).transpose(4, 0, 1, 3, 2)
            .reshape(P, 3 * KC * H)).astype(WDT_NP),
        "whh_t": np.ascontiguousarray(
            w_hh.reshape(3, KC, P, KC, P).transpose(4, 0, 1, 3, 2)
            .reshape(P, 3 * KC * H)).astype(WDT_NP),
        "brz": np.concatenate(
            [_pc((b_ih + b_hh)[:H]), _pc((b_ih + b_hh)[H : 2 * H])], axis=1),
        "b_in": _pc(b_ih[2 * H :]),
        "b_hn": _pc(b_hh[2 * H :]),
    }

    Wp = np.zeros((VP, H), np.float32)
    Wp[:V] = out_W
    bp = np.full(VP, NEG_BIG, np.float32)
    bp[:V] = out_b

    in_maps = []
    for i in range(N_CORES):
        vsl = slice(VS * i, VS * (i + 1))
        m = dict(rep)
        m["out_wt"] = np.ascontiguousarray(Wp[vsl].T.reshape(KC, P, VS)).astype(ODT_NP)
        m["out_b"] = np.ascontiguousarray(bp[vsl].reshape(VC, P).T)
        in_maps.append(m)
    return in_maps


LAST_RESULTS = None


def kernel(**inputs):
    global LAST_RESULTS
    nc = _get_nc()
    in_maps = make_in_maps(**inputs)
    trace = bool(int(os.environ.get("KERNEL_TRACE", "0")))
    res = bass_utils.run_bass_kernel_spmd(
        nc, in_maps, core_ids=list(range(N_CORES)), trace=trace,
    )
    LAST_RESULTS = res
    logits = np.concatenate(
        [np.asarray(r["logits"]).T.reshape(VS) for r in res.results])
    s_total = float(sum(np.asarray(r["ssum"]).reshape(()) for r in res.results))
    probs = (logits[:V] - np.float32(np.log(s_total))).reshape(1, V)
    h_new = np.asarray(res.results[0]["h_new"]).T.reshape(1, 1, H)
    return probs.astype(np.float32), h_new.astype(np.float32)


# revision 12
# speedup vs baseline: 2.0578x; 1.2474x over previous
"""Trainium2 Bass kernel for a single-step seq2seq GRU decoder with attention.

Computation (batch=1):
  embedded = emb[prev_word]                                      [1, H]
  attn_w   = softmax([embedded, h_prev] @ attn_W.T + attn_b)     [1, L]
  focused  = attn_w @ encoder_outputs                            [1, H]
  gru_in   = relu([focused, embedded] @ comb_W.T + comb_b)       [1, H]
  h_new    = GRU(gru_in, h_prev)                                 [1, H]
  probs    = log_softmax(h_new @ out_W.T + out_b)                [1, V]

Distribution over 8 NeuronCores, with NO cross-core collectives:
  - the vocab dim of out_W/out_b (the 206MB input that dominates the
    memory-bound roofline) is sharded 8 ways
  - the small attention/comb/GRU stages are replicated on every core
    (batch=1: cheaper than paying a cross-core sync for their shards)
  - log-softmax: each core emits its local exp-sum; the host unshard step
    combines the 8 scalars and subtracts log(S) while concatenating.
  Collectives are deliberately avoided: a NEFF with collectives pays a
  multi-core rendezvous at entry, which costs the full inter-core dispatch
  skew on every execution.

On-device layout: a hidden vector x[1024] lives as [128, 8] SBUF tiles with
x[c*128+p] at [p, c] (partition-parallel everywhere; no transposes needed).
"""

import os
import numpy as np
import ml_dtypes

import concourse.bass as bass
import concourse.bacc as bacc
import concourse.mybir as mybir
import concourse.tile as tile
import concourse.bass_utils as bass_utils

V, H, L = 50257, 1024, 20
N_CORES = 8
P = 128
KC = H // P            # 8 hidden chunks of 128
VP = 51200             # padded vocab = 8 * 6400
VS = VP // N_CORES     # 6400 vocab rows per core
VC = VS // P           # 50 vocab chunks of 128

F32 = mybir.dt.float32
BF16 = mybir.dt.bfloat16
NEG_BIG = -1.0e30

# dtype of the replicated comb/GRU weights (WDT) and the out_W shard (ODT).
# GRU stays bf16 (fp8 would push h_new error to ~1.5e-2); out_W tolerates
# fp8 e4m3 with a 256x scale (probs error ~3e-3 vs bf16's 3e-4).
WDT = BF16
WDT_NP = ml_dtypes.bfloat16
OUT_FP8 = True
ODT = mybir.dt.float8e4 if OUT_FP8 else BF16
ODT_NP = ml_dtypes.float8_e4m3 if OUT_FP8 else ml_dtypes.bfloat16
W_SCALE = 256.0 if OUT_FP8 else 1.0   # fp8 quantization scale for out_W
H_SCALE = 16.0 if OUT_FP8 else 1.0    # fp8 quantization scale for h_new rhs
UNSCALE = 1.0 / (W_SCALE * H_SCALE)


def build_nc():
    nc = bacc.Bacc(
        "TRN2",
        target_bir_lowering=False,
        debug=False,
        enable_asserts=False,
        num_devices=N_CORES,
    )

    def inp(name, shape, dt=F32):
        return nc.dram_tensor(name, shape, dt, kind="ExternalInput").ap()

    # replicated inputs
    emb_pc = inp("emb_pc", [P, KC])            # embedded word, (c p) -> p c
    hprev_pc = inp("hprev_pc", [P, KC])        # h_prev, (c p) -> p c
    attn_wt = inp("attn_wt", [P, 16 * L])      # attn_W.T chunked [p, c*L+m]
    attn_b = inp("attn_b", [L, 1])
    enc = inp("enc", [L, H])                   # encoder_outputs
    comb_wt = inp("comb_wt", [P, 16 * KC * P], WDT)  # [p,(co*16+ck)*128+m]
    comb_b = inp("comb_b", [P, KC])
    wih_t = inp("wih_t", [P, 3 * KC * H], WDT)  # [p,((g*8+co)*8+kc)*128+m]
    whh_t = inp("whh_t", [P, 3 * KC * H], WDT)
    brz = inp("brz", [P, 2 * KC])              # (b_ih+b_hh) r,z in (c p)
    b_in = inp("b_in", [P, KC])                # b_ih n slice
    b_hn = inp("b_hn", [P, KC])                # b_hh n slice
    # sharded inputs
    out_wt = inp("out_wt", [KC, P, VS], ODT)   # out_W shard .T chunks
    out_b = inp("out_b", [P, VC])              # bias shard [p, vc]

    logits_out = nc.dram_tensor("logits", [P, VC], F32, kind="ExternalOutput").ap()
    ssum_out = nc.dram_tensor("ssum", [1, 1], F32, kind="ExternalOutput").ap()
    hnew_out = nc.dram_tensor("h_new", [P, KC], F32, kind="ExternalOutput").ap()

    with tile.TileContext(nc) as tc:
        with (
            tc.tile_pool(name="consts", bufs=1) as consts,
            tc.tile_pool(name="sb", bufs=1) as sb,
            tc.tile_pool(name="wpool", bufs=1) as wpool,
            tc.tile_pool(name="psum", bufs=1, space="PSUM") as psum,
        ):
            ones = consts.tile([P, P], F32)
            nc.vector.memset(ones[:], 1.0)

            # ---- small/critical weights first (DMA order matters) --------
            emb_sb = sb.tile([P, KC], F32)
            nc.sync.dma_start(emb_sb[:], emb_pc)
            hprev_sb = sb.tile([P, KC], F32)
            nc.sync.dma_start(hprev_sb[:], hprev_pc)
            attn_wt_sb = sb.tile([P, 16 * L], F32)
            nc.sync.dma_start(attn_wt_sb[:], attn_wt)
            attn_b_sb = sb.tile([L, 1], F32)
            nc.sync.dma_start(attn_b_sb[:], attn_b)
            enc_sb = sb.tile([L, H], F32)
            nc.sync.dma_start(enc_sb[:], enc)
            comb_b_sb = sb.tile([P, KC], F32)
            nc.sync.dma_start(comb_b_sb[:], comb_b)
            brz_sb = sb.tile([P, 2 * KC], F32)
            nc.sync.dma_start(brz_sb[:], brz)
            b_in_sb = sb.tile([P, KC], F32)
            nc.sync.dma_start(b_in_sb[:], b_in)
            b_hn_sb = sb.tile([P, KC], F32)
            nc.sync.dma_start(b_hn_sb[:], b_hn)
            out_b_sb = sb.tile([P, VC], F32)
            nc.sync.dma_start(out_b_sb[:], out_b)

            comb_wt_sb = sb.tile([P, 16 * KC * P], WDT)
            nc.sync.dma_start(comb_wt_sb[:], comb_wt)
            wih_sb = sb.tile([P, 3 * KC * H], WDT)
            half = 3 * KC * H // 2
            nc.sync.dma_start(wih_sb[:, :half], wih_t[:, :half])
            nc.sync.dma_start(wih_sb[:, half:], wih_t[:, half:])
            whh_sb = sb.tile([P, 3 * KC * H], WDT)
            nc.sync.dma_start(whh_sb[:, :half], whh_t[:, :half])
            nc.sync.dma_start(whh_sb[:, half:], whh_t[:, half:])

            # ---- big out_W shard: 4-deep rotating stream of chunks -------
            # (all 8 resident would not fit next to the replicated GRU
            # weights; DMA refills slots as PE drains them)
            def load_w_chunk(kc):
                wt = wpool.tile([P, VS], ODT, tag="w", bufs=4)
                vh = VS // 2
                nc.sync.dma_start(wt[:, :vh], out_wt[kc][:, :vh])
                nc.sync.dma_start(wt[:, vh:], out_wt[kc][:, vh:])
                return wt

            # ---- stage A: attention (replicated, fp32) -------------------
            attnlog_ps = psum.tile([L, 1], F32, tag="pa")
            for c in range(16):
                rhs = emb_sb[:, c : c + 1] if c < KC else hprev_sb[:, c - KC : c - KC + 1]
                nc.tensor.matmul(
                    attnlog_ps[:],
                    attn_wt_sb[:, c * L : (c + 1) * L],
                    rhs,
                    start=(c == 0),
                    stop=(c == 15),
                )
            # exp(logit + b); logits are tiny so no max-subtraction needed
            expw_sb = sb.tile([L, 1], F32)
            nc.scalar.activation(
                expw_sb[:], attnlog_ps[:], mybir.ActivationFunctionType.Exp,
                bias=attn_b_sb[:],
            )
            asum_ps = psum.tile([1, 1], F32, tag="pb")
            nc.tensor.matmul(asum_ps[:], expw_sb[:], ones[:L, 0:1], start=True, stop=True)
            arecip_sb = sb.tile([1, 1], F32)
            nc.vector.reciprocal(arecip_sb[:], asum_ps[:])
            # focused (unnormalized) [128, KC]
            foc_ps = psum.tile([P, KC], F32, tag="pm")
            for c in range(KC):
                nc.tensor.matmul(
                    foc_ps[:, c : c + 1],
                    enc_sb[:, c * P : (c + 1) * P],
                    expw_sb[:],
                    start=True,
                    stop=True,
                )
            # broadcast 1/denom across partitions via PE ones column
            arb_ps = psum.tile([P, 1], F32, tag="pb")
            nc.tensor.matmul(arb_ps[:], ones[0:1, :], arecip_sb[:], start=True, stop=True)
            arb_sb = sb.tile([P, 1], F32)
            nc.scalar.copy(arb_sb[:], arb_ps[:])
            fsc_sb = sb.tile([P, KC], WDT)
            nc.vector.tensor_scalar_mul(fsc_sb[:], foc_ps[:], arb_sb[:])
            emb_w_sb = sb.tile([P, KC], WDT)
            nc.vector.tensor_copy(emb_w_sb[:], emb_sb[:])

            # ---- stage B: comb (replicated) -> gru_in [128, KC] ----------
            gcol_ps = psum.tile([P, KC], F32, tag="pm")
            for co in range(KC):
                for ck in range(16):
                    rhs = (fsc_sb[:, ck : ck + 1] if ck < KC
                           else emb_w_sb[:, ck - KC : ck - KC + 1])
                    nc.tensor.matmul(
                        gcol_ps[:, co : co + 1],
                        comb_wt_sb[:, (co * 16 + ck) * P : (co * 16 + ck + 1) * P],
                        rhs,
                        start=(ck == 0),
                        stop=(ck == 15),
                    )
            gin_f_sb = sb.tile([P, KC], F32)
            nc.vector.tensor_add(gin_f_sb[:], gcol_ps[:], comb_b_sb[:])
            nc.vector.tensor_relu(gin_f_sb[:], gin_f_sb[:])
            gin_sb = sb.tile([P, KC], WDT)
            nc.vector.tensor_copy(gin_sb[:], gin_f_sb[:])
            hprev_w_sb = sb.tile([P, KC], WDT)
            nc.vector.tensor_copy(hprev_w_sb[:], hprev_sb[:])

            # ---- stage C: GRU (replicated) -> h_new [128, KC] ------------
            # gi/gh columns j = g*KC + co (gate-major)
            gi_ps = psum.tile([P, 3 * KC], F32, tag="pgi")
            gh_ps = psum.tile([P, 3 * KC], F32, tag="pgh")
            for j in range(3 * KC):
                for kc in range(KC):
                    off = (j * KC + kc) * P
                    nc.tensor.matmul(
                        gi_ps[:, j : j + 1],
                        wih_sb[:, off : off + P],
                        gin_sb[:, kc : kc + 1],
                        start=(kc == 0),
                        stop=(kc == KC - 1),
                    )
            for j in range(3 * KC):
                for kc in range(KC):
                    off = (j * KC + kc) * P
                    nc.tensor.matmul(
                        gh_ps[:, j : j + 1],
                        whh_sb[:, off : off + P],
                        hprev_w_sb[:, kc : kc + 1],
                        start=(kc == 0),
                        stop=(kc == KC - 1),
                    )
            gh_sb = sb.tile([P, 3 * KC], F32)
            nc.scalar.copy(gh_sb[:], gh_ps[:])
            # r,z = sigmoid(gi + gh + brz) on the first 2*KC columns
            rz_sb = sb.tile([P, 2 * KC], F32)
            nc.vector.tensor_add(rz_sb[:], gi_ps[:, : 2 * KC], gh_sb[:, : 2 * KC])
            nc.vector.tensor_add(rz_sb[:], rz_sb[:], brz_sb[:])
            nc.scalar.activation(rz_sb[:], rz_sb[:], mybir.ActivationFunctionType.Sigmoid)
            # n = tanh(gi_n + b_in + r*(gh_n + b_hn))
            hnb_sb = sb.tile([P, KC], F32)
            nc.vector.tensor_add(hnb_sb[:], gh_sb[:, 2 * KC :], b_hn_sb[:])
            nc.vector.tensor_mul(hnb_sb[:], hnb_sb[:], rz_sb[:, :KC])
            npre_sb = sb.tile([P, KC], F32)
            nc.vector.tensor_add(npre_sb[:], gi_ps[:, 2 * KC :], hnb_sb[:])
            nc.vector.tensor_add(npre_sb[:], npre_sb[:], b_in_sb[:])
            n_sb = sb.tile([P, KC], F32)
            nc.scalar.activation(n_sb[:], npre_sb[:], mybir.ActivationFunctionType.Tanh)
            # h_new = n + z*(h - n)
            hmn_sb = sb.tile([P, KC], F32)
            nc.vector.tensor_sub(hmn_sb[:], hprev_sb[:], n_sb[:])
            nc.vector.tensor_mul(hmn_sb[:], hmn_sb[:], rz_sb[:, KC : 2 * KC])
            hnew_sb = sb.tile([P, KC], F32)
            nc.vector.tensor_add(hnew_sb[:], n_sb[:], hmn_sb[:])
            nc.sync.dma_start(hnew_out, hnew_sb[:])
            hnew_w_sb = sb.tile([P, KC], ODT)
            if OUT_FP8:
                nc.vector.tensor_scalar_mul(hnew_w_sb[:], hnew_sb[:], H_SCALE)
            else:
                nc.vector.tensor_copy(hnew_w_sb[:], hnew_sb[:])

            # ---- stage D: vocab projection shard -------------------------
            # Per hidden-chunk partials in PSUM, accumulated into SBUF by
            # DVE: PE starts on each weight chunk as its DMA lands, no
            # cross-chunk PSUM accumulation groups.
            logits_sb = sb.tile([P, VC], F32)
            for kc in range(KC):
                wt = load_w_chunk(kc)
                part_ps = psum.tile([P, VC], F32, tag="plog", bufs=2)
                for vc in range(VC):
                    nc.tensor.matmul(
                        part_ps[:, vc : vc + 1],
                        wt[:, vc * P : (vc + 1) * P],
                        hnew_w_sb[:, kc : kc + 1],
                        start=True,
                        stop=True,
                    )
                if kc == 0:
                    nc.vector.scalar_tensor_tensor(
                        logits_sb[:], part_ps[:], UNSCALE, out_b_sb[:],
                        op0=mybir.AluOpType.mult, op1=mybir.AluOpType.add,
                    )
                else:
                    nc.vector.scalar_tensor_tensor(
                        logits_sb[:], part_ps[:], UNSCALE, logits_sb[:],
                        op0=mybir.AluOpType.mult, op1=mybir.AluOpType.add,
                    )
            nc.sync.dma_start(logits_out, logits_sb[:])

            # ---- local exp-sum for the host-side log-softmax -------------
            exp_sb = sb.tile([P, VC], F32)
            srow_sb = sb.tile([P, 1], F32)
            nc.scalar.activation(
                exp_sb[:], logits_sb[:], mybir.ActivationFunctionType.Exp,
                accum_out=srow_sb[:],
            )
            ssum_ps = psum.tile([1, 1], F32, tag="pb")
            nc.tensor.matmul(ssum_ps[:], srow_sb[:], ones[:, 0:1], start=True, stop=True)
            ssum_sb = sb.tile([1, 1], F32)
            nc.scalar.copy(ssum_sb[:], ssum_ps[:])
            nc.sync.dma_start(ssum_out, ssum_sb[:])

    nc.compile()
    return nc


_NC_CACHE = None


def _get_nc():
    global _NC_CACHE
    if _NC_CACHE is None:
        _NC_CACHE = build_nc()
    return _NC_CACHE


def _pc(v):
    """[1024] -> [128, 8] with v[c*128+p] at [p, c]."""
    return np.ascontiguousarray(v.reshape(KC, P).T)


def make_in_maps(prev_word, prev_hidden, encoder_outputs, emb, attn_W, attn_b,
                 comb_W, comb_b, w_ih, w_hh, b_ih, b_hh, out_W, out_b):
    f32 = lambda a: np.asarray(a, dtype=np.float32)
    idx = int(np.asarray(prev_word).reshape(-1)[0])
    emb_row = f32(emb)[idx].reshape(H)
    hprev = f32(prev_hidden).reshape(H)
    attn_W = f32(attn_W)
    attn_b = f32(attn_b)
    enc = np.ascontiguousarray(f32(encoder_outputs))
    comb_W = f32(comb_W)
    comb_b = f32(comb_b)
    w_ih, w_hh, b_ih, b_hh = f32(w_ih), f32(w_hh), f32(b_ih), f32(b_hh)
    out_W, out_b = f32(out_W), f32(out_b)

    # replicated tensors (same arrays for every core)
    rep = {
        "emb_pc": _pc(emb_row),
        "hprev_pc": _pc(hprev),
        # [p, c*L+m] = attn_W[m, c*128+p]
        "attn_wt": np.ascontiguousarray(
            attn_W.T.reshape(16, P, L).transpose(1, 0, 2).reshape(P, 16 * L)),
        "attn_b": np.ascontiguousarray(attn_b.reshape(L, 1)),
        "enc": enc,
        # [p, (co*16+ck)*128+m] = comb_W[co*128+m, ck*128+p]
        "comb_wt": np.ascontiguousarray(
            comb_W.reshape(KC, P, 16, P).transpose(3, 0, 2, 1)
            .reshape(P, 16 * KC * P)).astype(WDT_NP),
        "comb_b": _pc(comb_b),
        # [p, ((g*8+co)*8+kc)*128+m] = W[g*1024+co*128+m, kc*128+p]
        "wih_t": np.ascontiguousarray(
            w_ih.reshape(3, KC, P, KC, P).transpose(4, 0, 1, 3, 2)
            .reshape(P, 3 * KC * H)).astype(WDT_NP),
        "whh_t": np.ascontiguousarray(
            w_hh.reshape(3, KC, P, KC, P).transpose(4, 0, 1, 3, 2)
            .reshape(P, 3 * KC * H)).astype(WDT_NP),
        "brz": np.concatenate(
            [_pc((b_ih + b_hh)[:H]), _pc((b_ih + b_hh)[H : 2 * H])], axis=1),
        "b_in": _pc(b_ih[2 * H :]),
        "b_hn": _pc(b_hh[2 * H :]),
    }

    Wp = np.zeros((VP, H), np.float32)
    Wp[:V] = out_W
    bp = np.full(VP, NEG_BIG, np.float32)
    bp[:V] = out_b

    in_maps = []
    for i in range(N_CORES):
        vsl = slice(VS * i, VS * (i + 1))
        m = dict(rep)
        m["out_wt"] = np.ascontiguousarray(
            np.clip(Wp[vsl].T.reshape(KC, P, VS) * W_SCALE, -240.0, 240.0)
        ).astype(ODT_NP)
        m["out_b"] = np.ascontiguousarray(bp[vsl].reshape(VC, P).T)
        in_maps.append(m)
    return in_maps


LAST_RESULTS = None


def kernel(**inputs):
    global LAST_RESULTS
    nc = _get_nc()
    in_maps = make_in_maps(**inputs)
    trace = bool(int(os.environ.get("KERNEL_TRACE", "0")))
    res = bass_utils.run_bass_kernel_spmd(
        nc, in_maps, core_ids=list(range(N_CORES)), trace=trace,
    )
    LAST_RESULTS = res
    logits = np.concatenate(
        [np.asarray(r["logits"]).T.reshape(VS) for r in res.results])
    s_total = float(sum(np.asarray(r["ssum"]).reshape(()) for r in res.results))
    probs = (logits[:V] - np.float32(np.log(s_total))).reshape(1, V)
    h_new = np.asarray(res.results[0]["h_new"]).T.reshape(1, 1, H)
    return probs.astype(np.float32), h_new.astype(np.float32)
